# revision 1
# baseline (speedup 1.0000x reference)
"""Trainium2 Bass kernel for nn_ClothGraphConvNetwork_MLPDecoder.

8 NeuronCores, data-parallel over batch (2 batches/core), no collectives.

Design:
- Channels-major activations (channels on SBUF partitions, vertices on free).
- lin0 on the broadcast-concat input is separable: x0 = U + v_b with
  U = W0v @ verts (batch-independent, regenerated on the fly, K=4) and
  v_b = W0img @ img[b] (per-channel constant). The 116 GFLOP dense lin0 is
  never executed; its GroupNorm stats come analytically from per-channel
  U stats (bn_stats) plus v_b.
- Graph conv: sup = y^T @ cW leaves the PE vertex-major; the edge-list
  scatter-add is a dense matmul against a host-built adjacency matrix
  AT[src, dst] (bf16, SBUF-resident), landing channels-major again. Both
  steps are pure PE work - no transposes, no gather DMAs.
- GroupNorm (every instance has group size 8, adjacent channels): bn_stats
  per channel -> one batched chain per GN instance: 8-to-1 group-mean via a
  tiny PE matmul with a 1/8-indicator matrix over all tiles' [m, E2]
  columns at once -> sqrt/reciprocal -> broadcast back via a second tiny
  matmul -> per-tile ACT pass relu(a*x + beta) with per-partition
  scale/bias.
- dtypes: big matmuls float32r (full PE rate), adjacency matmul bf16,
  tiny stat matmuls fp32, activations stored f32r.
- fp32r ISA restrictions: even matmul free dims -> vertices padded
  1723 -> 1724, F-chunks (432,432,432,428), sup M-tiles 128/60.
"""

import contextlib

import numpy as np
import ml_dtypes

import concourse.bass as bass
import concourse.tile as tile
from concourse import bacc, mybir
from concourse.bass_utils import run_bass_kernel_spmd

F32R = mybir.dt.float32r
F32 = mybir.dt.float32
BF16 = mybir.dt.bfloat16
AF = mybir.ActivationFunctionType
ALU = mybir.AluOpType

B, N, DEG = 16, 1723, 8
C, L, H = 512, 5, 256
NP = 1724              # padded vertex count (even, fp32r requirement)
NCORES = 8
BLOC = B // NCORES     # batches per core
NT = 14                # vertex 128-tiles (last has 59 real rows)
FCH = [(0, 432), (432, 432), (864, 432), (1296, 428)]    # even chunks of NP
BNCH = [(0, 512), (512, 1024), (1024, 1536), (1536, N)]  # bn_stats chunks of N


def _param_layout():
    items = [("lin0_b", 1024),
             ("b0_pre_g", 1024), ("b0_pre_b", 1024),
             ("b0_lin1_b", 256), ("b0_n1_g", 256), ("b0_n1_b", 256),
             ("b0_conv_b", 256), ("b0_n2_g", 256), ("b0_n2_b", 256),
             ("b0_lin2_b", 512), ("b0_skip_b", 512)]
    for i in range(L):
        items += [(f"blk_pre_g{i}", 512), (f"blk_pre_b{i}", 512),
                  (f"blk_lin1_b{i}", 256), (f"blk_n1_g{i}", 256),
                  (f"blk_n1_b{i}", 256), (f"blk_conv_b{i}", 256),
                  (f"blk_n2_g{i}", 256), (f"blk_n2_b{i}", 256),
                  (f"blk_lin2_b{i}", 512)]
    items += [("h1_b", 64), ("h2_b", 32), ("hn_g", 32), ("hn_b", 32),
              ("h3_b", 3)]
    idx = {}
    pos = 0
    for name, ln in items:
        for t in range((ln + 127) // 128):
            idx[(name, t)] = pos
            pos += 1
    return items, idx, pos


PARAM_ITEMS, PIDX, NSLOT = _param_layout()
PHASES = []


def build(nreps=1, no_gn=False, no_agg=False, no_bn=False):
    nc = bacc.Bacc("TRN2", target_bir_lowering=False, debug=False)
    PHASES.clear()

    def _mark(label):
        PHASES.append((label, nc.next_id()))

    d = {}

    def din(name, shape, dt):
        d[name] = nc.dram_tensor(name, list(shape), dt, kind="ExternalInput")

    din("verts", (4, NP), F32R)
    din("at", (NT, 128, NP), BF16)
    din("w0vt", (4, 1024), F32R)
    din("w0imgt", (16, 128, 1024), F32R)
    din("imgt", (16, 128, BLOC), F32R)
    din("g8", (128, 16), F32)     # indicator / 8  (group-mean reduce)
    din("g8t", (16, 128), F32)    # 0/1 indicator transpose (broadcast)
    din("ident", (128, 128), F32)
    din("identr", (128, 128), F32R)
    din("prm", (128, NSLOT), F32)
    din("b0l1t", (8, 128, H), F32R)
    din("b0cw", (2, 128, H), F32R)
    din("b0l2t", (2, 128, C), F32R)
    din("b0skt", (8, 128, C), F32R)
    din("bl1t", (L, 4, 128, H), F32R)
    din("bcw", (L, 2, 128, H), F32R)
    din("bl2t", (L, 2, 128, C), F32R)
    din("h1t", (4, 128, 64), F32R)
    din("h2t", (64, 32), F32R)
    din("h3t", (32, 4), F32R)
    out_d = nc.dram_tensor("out", [BLOC, 3, N], F32, kind="ExternalOutput")

    with tile.TileContext(nc) as tc, contextlib.ExitStack() as ctx:
        cons = ctx.enter_context(tc.tile_pool(name="cons", bufs=1))
        ps = ctx.enter_context(tc.tile_pool(name="ps", bufs=8, space="PSUM"))
        sm = ctx.enter_context(tc.tile_pool(name="sm", bufs=2))
        xp = ctx.enter_context(tc.tile_pool(name="xp", bufs=4))
        yp = ctx.enter_context(tc.tile_pool(name="yp", bufs=3))
        supp = ctx.enter_context(tc.tile_pool(name="supp", bufs=1))
        xrp = ctx.enter_context(tc.tile_pool(name="xrp", bufs=3))
        tmpp = ctx.enter_context(tc.tile_pool(name="tmpp", bufs=2))
        yhp = ctx.enter_context(tc.tile_pool(name="yhp", bufs=2))
        bwp = ctx.enter_context(tc.tile_pool(name="bwp", bufs=2))
        hwp = ctx.enter_context(tc.tile_pool(name="hwp", bufs=1))

        # ---- constants ----
        g8 = cons.tile([128, 16], F32)
        nc.sync.dma_start(g8[:], d["g8"].ap())
        g8t = cons.tile([16, 128], F32)
        nc.sync.dma_start(g8t[:], d["g8t"].ap())
        ident = cons.tile([128, 128], F32)
        nc.sync.dma_start(ident[:], d["ident"].ap())
        identr = cons.tile([128, 128], F32R)
        nc.sync.dma_start(identr[:], d["identr"].ap())
        eps = cons.tile([128, 1], F32)
        nc.vector.memset(eps[:], 1e-5)
        prm = cons.tile([128, NSLOT], F32)
        nc.sync.dma_start(prm[:], d["prm"].ap())
        verts = cons.tile([4, NP], F32R)
        nc.sync.dma_start(verts[:], d["verts"].ap())
        w0vt = cons.tile([4, 1024], F32R)
        nc.sync.dma_start(w0vt[:], d["w0vt"].ap())
        asb = cons.tile([128, NT, NP], BF16)
        _dmae = [nc.sync, nc.gpsimd]
        for kt in range(NT):
            _dmae[kt % 2].dma_start(asb[:, kt, :], d["at"].ap()[kt])
        cw0 = cons.tile([128, 2, H], F32R)
        for ct in range(2):
            nc.sync.dma_start(cw0[:, ct, :], d["b0cw"].ap()[ct])
        l2t0 = cons.tile([128, 2, C], F32R)
        for ct in range(2):
            nc.sync.dma_start(l2t0[:, ct, :], d["b0l2t"].ap()[ct])
        h1w = hwp.tile([128, 4, 64], F32R)
        for kt in range(4):
            nc.sync.dma_start(h1w[:, kt, :], d["h1t"].ap()[kt])
        h2w = hwp.tile([64, 32], F32R)
        nc.sync.dma_start(h2w[:], d["h2t"].ap())
        h3w = hwp.tile([32, 4], F32R)
        nc.sync.dma_start(h3w[:], d["h3t"].ap())
        abc = cons.tile([128, 8, 2], F32)
        nc.vector.memset(abc[:, :, 0:1], 1.0)
        nc.vector.memset(abc[:, :, 1:2], 0.0)

        def P(name, t=0, parts=128, width=1):
            i = PIDX[(name, t)]
            return prm[0:parts, i:i + width]

        def gn_chain(st3, gname, bname, T, parts=128, G=16,
                     abtag="ab", abbufs=2):
            """Batched per-instance GN: st3 (parts, T, 2) holds per-channel
            [mean, E2] for T channel-tiles. Returns ab (128, T, 2) with
            per-channel [a, beta]."""
            if no_gn:
                return abc
            psg = ps.tile([16, 8, 2], F32, tag="ps", name="psg")
            nc.tensor.matmul(psg[0:G, 0:T, :], g8[0:parts, 0:G],
                             st3[0:parts, 0:T, :], start=True, stop=True)
            pg = sm.tile([16, 8, 2], F32, tag="pg", bufs=3, name="pg")
            nc.vector.tensor_copy(pg[0:G, 0:T, :], psg[0:G, 0:T, :])
            t2 = sm.tile([16, 8], F32, tag="t2", bufs=3, name="t2")
            nc.vector.tensor_tensor(t2[0:G, 0:T], pg[0:G, 0:T, 0],
                                    pg[0:G, 0:T, 0], op=ALU.mult)
            nc.vector.tensor_tensor(t2[0:G, 0:T], pg[0:G, 0:T, 1],
                                    t2[0:G, 0:T], op=ALU.subtract)
            nc.scalar.activation(t2[0:G, 0:T], t2[0:G, 0:T], AF.Sqrt,
                                 bias=eps[0:G, :])
            mr = sm.tile([16, 8, 2], F32, tag="mr", bufs=3, name="mr")
            nc.vector.tensor_copy(mr[0:G, 0:T, 0], pg[0:G, 0:T, 0])
            nc.vector.reciprocal(mr[0:G, 0:T, 1], t2[0:G, 0:T])
            psb = ps.tile([128, 8, 2], F32, tag="ps", name="psb")
            nc.tensor.matmul(psb[0:parts, 0:T, :], g8t[0:G, 0:parts],
                             mr[0:G, 0:T, :], start=True, stop=True)
            ab = sm.tile([128, 8, 2], F32, tag=abtag, bufs=abbufs, name="ab")
            nc.vector.tensor_tensor(ab[0:parts, 0:T, 0], psb[0:parts, 0:T, 1],
                                    P(gname, 0, parts, T), op=ALU.mult)
            t3 = sm.tile([128, 8], F32, tag="t3", bufs=3, name="t3")
            nc.vector.tensor_tensor(t3[0:parts, 0:T], psb[0:parts, 0:T, 0],
                                    ab[0:parts, 0:T, 0], op=ALU.mult)
            nc.vector.tensor_tensor(ab[0:parts, 0:T, 1],
                                    P(bname, 0, parts, T),
                                    t3[0:parts, 0:T], op=ALU.subtract)
            return ab

        def stats_new(T):
            return [sm.tile([128, 4, 6], F32, tag="stats", bufs=10,
                            name="sts") for _ in range(T)]

        def note(stt, ci, x_ap, f0, fw, parts=128):
            """Record bn_stats for one freshly-written chunk (pad excluded)."""
            if no_gn and no_bn:
                return
            rw = fw if f0 + fw <= N else (N - f0)
            nc.vector.bn_stats(stt[0:parts, ci, :],
                               x_ap[0:parts, f0:f0 + rw])

        def gn_finish(stats_list, gname, bname, parts=128, G=16,
                      abtag="ab", abbufs=2):
            if no_gn and no_bn:
                return abc
            T = len(stats_list)
            st3 = sm.tile([128, 8, 2], F32, tag="st3", bufs=2, name="st3")
            for t, stt in enumerate(stats_list):
                nc.vector.bn_aggr(st3[0:parts, t, :], stt[0:parts, :, :])
            sq = sm.tile([128, 8], F32, tag="sq", bufs=2, name="sq")
            nc.vector.tensor_tensor(sq[0:parts, 0:T], st3[0:parts, 0:T, 0],
                                    st3[0:parts, 0:T, 0], op=ALU.mult)
            nc.vector.tensor_tensor(st3[0:parts, 0:T, 1],
                                    st3[0:parts, 0:T, 1],
                                    sq[0:parts, 0:T], op=ALU.add)
            return gn_chain(st3, gname, bname, T, parts=parts, G=G,
                            abtag=abtag, abbufs=abbufs)

        def gn_f1a(stt, parts=128, G=16):
            """Stage A of a single-tile GN chain: aggregate + group-mean
            matmul + rsqrt; returns mr (G, 2) = [m_g, rs_g]."""
            st3 = sm.tile([128, 1, 2], F32, tag="st1", bufs=6, name="st1")
            nc.vector.bn_aggr(st3[0:parts, 0, :], stt[0:parts, :, :])
            sq = sm.tile([128, 1], F32, tag="sq1", bufs=6, name="sq1")
            nc.vector.tensor_tensor(sq[0:parts, :], st3[0:parts, 0, 0:1],
                                    st3[0:parts, 0, 0:1], op=ALU.mult)
            nc.vector.tensor_tensor(st3[0:parts, 0, 1:2],
                                    st3[0:parts, 0, 1:2],
                                    sq[0:parts, :], op=ALU.add)
            psg = ps.tile([16, 2], F32, tag="ps", name="psg1")
            nc.tensor.matmul(psg[0:G, :], g8[0:parts, 0:G],
                             st3[0:parts, 0, :], start=True, stop=True)
            pg = sm.tile([16, 2], F32, tag="pg1", bufs=6, name="pg1")
            nc.vector.tensor_copy(pg[0:G, :], psg[0:G, :])
            t2 = sm.tile([16, 1], F32, tag="t21", bufs=6, name="t21")
            nc.vector.tensor_tensor(t2[0:G, :], pg[0:G, 0:1], pg[0:G, 0:1],
                                    op=ALU.mult)
            nc.vector.tensor_tensor(t2[0:G, :], pg[0:G, 1:2], t2[0:G, :],
                                    op=ALU.subtract)
            nc.scalar.activation(t2[0:G, :], t2[0:G, :], AF.Sqrt,
                                 bias=eps[0:G, :])
            mr = sm.tile([16, 2], F32, tag="mr1", bufs=6, name="mr1")
            nc.vector.tensor_copy(mr[0:G, 0:1], pg[0:G, 0:1])
            nc.vector.reciprocal(mr[0:G, 1:2], t2[0:G, :])
            return mr

        def gn_f1b(mr, gname, gt, bname, parts=128, G=16,
                   abtag="ab", abbufs=4):
            """Stage B: broadcast matmul + per-channel [a, beta]."""
            psb = ps.tile([128, 2], F32, tag="ps", name="psb1")
            nc.tensor.matmul(psb[0:parts, :], g8t[0:G, 0:parts], mr[0:G, :],
                             start=True, stop=True)
            ab = sm.tile([128, 2], F32, tag=abtag, bufs=abbufs, name="ab1")
            nc.vector.tensor_tensor(ab[0:parts, 0:1], psb[0:parts, 1:2],
                                    P(gname, gt, parts), op=ALU.mult)
            t3 = sm.tile([128, 1], F32, tag="t31", bufs=6, name="t31")
            nc.vector.tensor_tensor(t3[0:parts, :], psb[0:parts, 0:1],
                                    ab[0:parts, 0:1], op=ALU.mult)
            nc.vector.tensor_tensor(ab[0:parts, 1:2], P(bname, gt, parts),
                                    t3[0:parts, :], op=ALU.subtract)
            return ab

        def gn_finish1(stt, gname, gt, bname, parts=128, G=16,
                       abtag="ab", abbufs=4):
            if no_gn and no_bn:
                return abc
            mr = gn_f1a(stt, parts=parts, G=G)
            return gn_f1b(mr, gname, gt, bname, parts=parts, G=G,
                          abtag=abtag, abbufs=abbufs)

        def block_tail(x_tiles, y1, pn, cw_t, l2_t, ystats, pn_next):
            """GN2 -> sup -> adjacency matmul -> GN3 -> lin2 -> residual.

            ystats: per-y1-tile chunk stats collected during lin1 evacuation.
            pn_next: param-name fn of the NEXT block; when set, chunk stats of
            the updated x are collected and each tile's GN1 coefficient chain
            is emitted as soon as that tile is final (so it hides under the
            remaining lin2 waves). Returns the list of [a, beta] tiles."""
            collect_x = pn_next is not None
            _mark("gn2")
            for ct in range(2):
                ab = gn_finish1(ystats[ct], pn("n1_g"), ct, pn("n1_b"))
                for (f0, fw) in FCH:
                    nc.scalar.activation(y1[ct][:, f0:f0 + fw],
                                         y1[ct][:, f0:f0 + fw], AF.Relu,
                                         bias=ab[:, 1:2],
                                         scale=ab[:, 0:1])
            _mark("sup")
            sup = supp.tile([128, NT, H], BF16, tag="sup", name="sup")
            for half in (range(0, 7), range(7, NT)):
                spss = {}
                for ct in range(2):
                    for nt in half:
                        ms = nt * 128
                        mw = min(ms + 128, NP) - ms
                        if ct == 0:
                            spss[nt] = ps.tile([128, H], F32, tag="ps",
                                               name="sps")
                        nc.tensor.matmul(spss[nt][0:mw, :],
                                         y1[ct][:, ms:ms + mw], cw_t(ct),
                                         start=(ct == 0), stop=(ct == 1))
                for nt in half:
                    ms = nt * 128
                    mw = min(ms + 128, NP) - ms
                    if nt % 2 == 0:
                        nc.vector.tensor_copy(sup[0:mw, nt, :],
                                              spss[nt][0:mw, :])
                    else:
                        nc.scalar.copy(sup[0:mw, nt, :], spss[nt][0:mw, :])
            y2 = [yp.tile([128, NP], F32R, tag="y", name="y2") for _ in range(2)]
            _mark("agg")
            y2stats = stats_new(2)
            for dt in range(2):
                for ci, (f0, fw) in enumerate(FCH):
                    aps = ps.tile([128, 512], F32, tag="ps", name="aps")
                    for kt in range(1 if no_agg else NT):
                        kn = min(128, N - kt * 128)
                        nc.tensor.matmul(
                            aps[:, :fw],
                            sup[0:kn, kt, dt * 128:(dt + 1) * 128],
                            asb[0:kn, kt, f0:f0 + fw],
                            start=(kt == 0),
                            stop=(kt == (0 if no_agg else NT - 1)))
                    nc.scalar.activation(y2[dt][:, f0:f0 + fw], aps[:, :fw],
                                         AF.Identity, bias=P(pn("conv_b"), dt))
                    note(y2stats[dt], ci, y2[dt], f0, fw)
                if dt == 0:
                    # dt0's chain + apply hide under dt1's adjacency matmuls
                    _mark("gn3")
                    ab = gn_finish1(y2stats[0], pn("n2_g"), 0, pn("n2_b"))
                    for (f0, fw) in FCH:
                        nc.scalar.activation(y2[0][:, f0:f0 + fw],
                                             y2[0][:, f0:f0 + fw], AF.Relu,
                                             bias=ab[:, 1:2],
                                             scale=ab[:, 0:1])
            _mark("gn3")
            mr1 = gn_f1a(y2stats[1])
            _mark("lin2")
            xstats = stats_new(4) if collect_x else None
            xab = [None] * 4
            pre = [(0, 0), (0, 1), (0, 2), (0, 3), (1, 0), (1, 1)]
            lps = {}
            # pre-start some residual identity steps: they depend only on x,
            # keeping the PE busy while the dt1 GN3 chain completes
            for (mt, ci) in pre:
                f0, fw = FCH[ci]
                lps[(mt, ci)] = ps.tile([128, 512], F32, tag="ps", name="lps")
                nc.tensor.matmul(lps[(mt, ci)][:, :fw], identr[:],
                                 x_tiles[mt][:, f0:f0 + fw],
                                 start=True, stop=False)
            ab = gn_f1b(mr1, pn("n2_g"), 1, pn("n2_b"))
            for (f0, fw) in FCH:
                nc.scalar.activation(y2[1][:, f0:f0 + fw],
                                     y2[1][:, f0:f0 + fw], AF.Relu,
                                     bias=ab[:, 1:2], scale=ab[:, 0:1])
            for mt in range(4):
                for ci, (f0, fw) in enumerate(FCH):
                    if (mt, ci) not in lps:
                        lps[(mt, ci)] = ps.tile([128, 512], F32, tag="ps",
                                                name="lps")
                        nc.tensor.matmul(lps[(mt, ci)][:, :fw], identr[:],
                                         x_tiles[mt][:, f0:f0 + fw],
                                         start=True, stop=False)
                    for ct in range(2):
                        nc.tensor.matmul(lps[(mt, ci)][:, :fw],
                                         l2_t(ct, mt),
                                         y2[ct][:, f0:f0 + fw],
                                         start=False, stop=(ct == 1))
                    nc.scalar.activation(x_tiles[mt][:, f0:f0 + fw],
                                         lps[(mt, ci)][:, :fw],
                                         AF.Identity,
                                         bias=P(pn("lin2_b"), mt))
                    if collect_x:
                        note(xstats[mt], ci, x_tiles[mt], f0, fw)
                    del lps[(mt, ci)]
                if collect_x:
                    xab[mt] = gn_f1a(xstats[mt])
            if collect_x:
                for mt in range(4):
                    xab[mt] = gn_f1b(xab[mt], pn_next("pre_g"), mt,
                                     pn_next("pre_b"), abtag="abx", abbufs=4)
            return xab if collect_x else None

        rep = tc.For_i(0, nreps, 1) if nreps > 1 else contextlib.nullcontext()
        with rep:
            _mark("setup")
            # ---- setup: U per-channel stats (PE starts immediately) ----
            uch = cons.tile([128, 8, 2], F32, name="uch")
            for kt in range(8):
                ust = sm.tile([128, 4, 6], F32, tag="stats", bufs=10, name="ust")
                for ci, (f0, fw) in enumerate(FCH):
                    ups = ps.tile([128, 512], F32, tag="ps", name="ups")
                    nc.tensor.matmul(ups[:, :fw],
                                     w0vt[:, kt * 128:(kt + 1) * 128],
                                     verts[:, f0:f0 + fw], start=True,
                                     stop=True)
                    rw = fw if f0 + fw <= N else (N - f0)
                    nc.vector.bn_stats(ust[:, ci, :], ups[:, 0:rw])
                nc.vector.bn_aggr(uch[:, kt, :], ust[:, :, :])
            # ---- setup: v_b (bf16, single pass over W0img) ----
            vts = cons.tile([2, 1024], F32, name="vts")
            vps = [ps.tile([2, 512], F32, tag="ps", name="vps") for _ in range(2)]
            for kt in range(16):
                w0i = bwp.tile([128, 1024], F32R, tag="w0i", name="w0i")
                nc.sync.dma_start(w0i[:], d["w0imgt"].ap()[kt])
                img = bwp.tile([128, BLOC], F32R, tag="img", name="img")
                nc.gpsimd.dma_start(img[:], d["imgt"].ap()[kt])
                for half in range(2):
                    nc.tensor.matmul(vps[half][0:BLOC, :], img[:],
                                     w0i[:, half * 512:(half + 1) * 512],
                                     start=(kt == 0), stop=(kt == 15))
            for half in range(2):
                nc.scalar.activation(vts[:, half * 512:(half + 1) * 512],
                                     vps[half][0:BLOC, :], AF.Copy)
            vb = cons.tile([128, 8, BLOC], F32, name="vb")
            for mt in range(8):
                tps = ps.tile([128, BLOC], F32, tag="ps", name="tps")
                nc.tensor.transpose(tps[:, :], vts[:, mt * 128:(mt + 1) * 128],
                                    ident[0:BLOC, 0:BLOC])
                nc.scalar.activation(vb[:, mt, :], tps[:, :], AF.Identity,
                                     bias=P("lin0_b", mt))

            for b in range(BLOC):
                _mark("b0gn1")
                # ---- b0 GN1: analytic coefficients, batched over 8 tiles ----
                st3 = sm.tile([128, 8, 2], F32, tag="st3", bufs=2, name="st3b")
                sq = sm.tile([128, 8], F32, tag="sq", bufs=2, name="sqb")
                nc.vector.tensor_tensor(st3[:, :, 0], uch[:, :, 0],
                                        vb[:, :, b], op=ALU.add)
                nc.vector.tensor_tensor(sq[:, :], st3[:, :, 0], st3[:, :, 0],
                                        op=ALU.mult)
                nc.vector.tensor_tensor(st3[:, :, 1], uch[:, :, 1], sq[:, :],
                                        op=ALU.add)
                ab0 = gn_chain(st3, "b0_pre_g", "b0_pre_b", 8,
                               abtag="ab0", abbufs=2)
                if not no_gn:
                    t5 = sm.tile([128, 8], F32, tag="t5", bufs=2, name="t5")
                    nc.vector.tensor_tensor(t5[:, :], ab0[:, :, 0],
                                            vb[:, :, b], op=ALU.mult)
                    nc.vector.tensor_tensor(ab0[:, :, 1], ab0[:, :, 1],
                                            t5[:, :], op=ALU.add)

                # ---- b0 front ----
                _mark("b0front")
                x_tiles = [xp.tile([128, NP], F32R, tag="x", name="x")
                           for _ in range(4)]
                y1 = [yp.tile([128, NP], F32R, tag="y", name="y1")
                      for _ in range(2)]
                ystats = stats_new(2)
                for ci, (f0, fw) in enumerate(FCH):
                    y1ps = [ps.tile([128, 512], F32, tag="ps", name="y1ps")
                            for _ in range(2)]
                    xps = [ps.tile([128, 512], F32, tag="ps", name="xps")
                           for _ in range(4)]
                    for kt in range(8):
                        ups = ps.tile([128, 512], F32, tag="ps", name="ups2")
                        nc.tensor.matmul(ups[:, :fw],
                                         w0vt[:, kt * 128:(kt + 1) * 128],
                                         verts[:, f0:f0 + fw],
                                         start=True, stop=True)
                        x0r = xrp.tile([128, 432], F32R, tag="xr", name="x0r")
                        nc.scalar.activation(x0r[:, :fw], ups[:, :fw], AF.Relu,
                                             bias=ab0[:, kt, 1:2],
                                             scale=ab0[:, kt, 0:1])
                        # raw x0 chunk (U + v_b): reference's skip path input
                        x0c = xrp.tile([128, 432], F32R, tag="xc", name="x0c")
                        nc.vector.tensor_scalar(x0c[:, :fw], ups[:, :fw],
                                                vb[:, kt, b:b + 1], None,
                                                op0=ALU.add)
                        l1k = bwp.tile([128, H], F32R, tag="l1k", name="l1k")
                        nc.sync.dma_start(l1k[:], d["b0l1t"].ap()[kt])
                        skk = bwp.tile([128, C], F32R, tag="skk", name="skk")
                        nc.sync.dma_start(skk[:], d["b0skt"].ap()[kt])
                        for mt in range(2):
                            nc.tensor.matmul(y1ps[mt][:, :fw],
                                             l1k[:, mt * 128:(mt + 1) * 128],
                                             x0r[:, :fw],
                                             start=(kt == 0), stop=(kt == 7))
                        for mt in range(4):
                            nc.tensor.matmul(xps[mt][:, :fw],
                                             skk[:, mt * 128:(mt + 1) * 128],
                                             x0c[:, :fw],
                                             start=(kt == 0), stop=(kt == 7))
                    for mt in range(2):
                        nc.scalar.activation(y1[mt][:, f0:f0 + fw],
                                             y1ps[mt][:, :fw], AF.Identity,
                                             bias=P("b0_lin1_b", mt))
                        note(ystats[mt], ci, y1[mt], f0, fw)
                    for mt in range(4):
                        nc.scalar.activation(x_tiles[mt][:, f0:f0 + fw],
                                             xps[mt][:, :fw], AF.Identity,
                                             bias=P("b0_skip_b", mt))
                xab = block_tail(
                    x_tiles, y1, lambda s: "b0_" + s,
                    lambda ct: cw0[:, ct, :],
                    lambda ct, mt: l2t0[:, ct, mt * 128:(mt + 1) * 128],
                    ystats, pn_next=lambda s: f"blk_{s}0")

                # ---- 5 residual blocks ----
                for i in range(L):
                    bl1 = bwp.tile([128, 4, H], F32R, tag="bl1", name="bl1")
                    for ct in range(4):
                        nc.sync.dma_start(bl1[:, ct, :], d["bl1t"].ap()[i, ct])
                    bcw = bwp.tile([128, 2, H], F32R, tag="bcw", name="bcw")
                    for ct in range(2):
                        nc.sync.dma_start(bcw[:, ct, :], d["bcw"].ap()[i, ct])
                    bl2 = bwp.tile([128, 2, C], F32R, tag="bl2", name="bl2")
                    for ct in range(2):
                        nc.sync.dma_start(bl2[:, ct, :], d["bl2t"].ap()[i, ct])

                    _mark("blkgn1")
                    abx = xab
                    y1 = [yp.tile([128, NP], F32R, tag="y", name="y1b")
                          for _ in range(2)]
                    _mark("lin1")
                    ystats = stats_new(2)
                    for ci, (f0, fw) in enumerate(FCH):
                        y1ps = [ps.tile([128, 512], F32, tag="ps", name="y1psb")
                                for _ in range(2)]
                        for ct in range(4):
                            xr = xrp.tile([128, 432], F32R, tag="xr",
                                          name="xrb")
                            if ct % 2 == 0:
                                nc.scalar.activation(xr[:, :fw],
                                                     x_tiles[ct][:, f0:f0 + fw],
                                                     AF.Relu,
                                                     bias=abx[ct][:, 1:2],
                                                     scale=abx[ct][:, 0:1])
                            else:
                                nc.vector.tensor_scalar(
                                    xr[:, :fw], x_tiles[ct][:, f0:f0 + fw],
                                    abx[ct][:, 0:1], abx[ct][:, 1:2],
                                    op0=ALU.mult, op1=ALU.add)
                                nc.vector.tensor_scalar_max(
                                    xr[:, :fw], xr[:, :fw], 0.0)
                            for mt in range(2):
                                nc.tensor.matmul(
                                    y1ps[mt][:, :fw],
                                    bl1[:, ct, mt * 128:(mt + 1) * 128],
                                    xr[:, :fw],
                                    start=(ct == 0), stop=(ct == 3))
                        for mt in range(2):
                            nc.scalar.activation(y1[mt][:, f0:f0 + fw],
                                                 y1ps[mt][:, :fw], AF.Identity,
                                                 bias=P(f"blk_lin1_b{i}", mt))
                            note(ystats[mt], ci, y1[mt], f0, fw)
                    pn_next = (lambda s, j=i + 1: f"blk_{s}{j}") \
                        if i < L - 1 else None
                    xab = block_tail(
                        x_tiles, y1, lambda s, i=i: f"blk_{s}{i}",
                        lambda ct, _w=bcw: _w[:, ct, :],
                        lambda ct, mt, _w=bl2: _w[:, ct, mt * 128:(mt + 1) * 128],
                        ystats, pn_next=pn_next)

                # ---- head ----
                _mark("head")
                yh1 = yhp.tile([64, NP], F32R, tag="yh", name="yh1")
                for (f0, fw) in FCH:
                    hps = ps.tile([64, 512], F32, tag="ps", name="hps")
                    for kt in range(4):
                        nc.tensor.matmul(hps[:, :fw], h1w[:, kt, :],
                                         x_tiles[kt][:, f0:f0 + fw],
                                         start=(kt == 0), stop=(kt == 3))
                    nc.scalar.activation(yh1[:, f0:f0 + fw], hps[:, :fw],
                                         AF.Relu, bias=P("h1_b", 0, 64))
                yh2 = yhp.tile([32, NP], F32R, tag="yh", name="yh2")
                hstats = stats_new(1)
                for ci, (f0, fw) in enumerate(FCH):
                    hps2 = ps.tile([32, 512], F32, tag="ps", name="hps2")
                    nc.tensor.matmul(hps2[:, :fw], h2w[:], yh1[:, f0:f0 + fw],
                                     start=True, stop=True)
                    nc.scalar.activation(yh2[:, f0:f0 + fw], hps2[:, :fw],
                                         AF.Identity, bias=P("h2_b", 0, 32))
                    note(hstats[0], ci, yh2, f0, fw, parts=32)
                abh = gn_finish1(hstats[0], "hn_g", 0, "hn_b", parts=32, G=4)
                for (f0, fw) in FCH:
                    nc.scalar.activation(yh2[:, f0:f0 + fw],
                                         yh2[:, f0:f0 + fw], AF.Relu,
                                         bias=abh[0:32, 1:2],
                                         scale=abh[0:32, 0:1])
                osb = yhp.tile([4, NP], F32, tag="yh", name="osb")
                for (f0, fw) in FCH:
                    hps3 = ps.tile([4, 512], F32, tag="ps", name="hps3")
                    nc.tensor.matmul(hps3[:, :fw], h3w[:], yh2[:, f0:f0 + fw],
                                     start=True, stop=True)
                    nc.scalar.activation(osb[0:3, f0:f0 + fw], hps3[0:3, :fw],
                                         AF.Identity, bias=P("h3_b", 0, 3))
                nc.sync.dma_start(out_d.ap()[b], osb[0:3, 0:N])

    nc.compile()
    return nc


def _host_prep(inputs):
    f32 = np.float32
    shared = {}

    verts = np.zeros((4, NP), f32)
    verts[0:3, 0:N] = np.asarray(inputs["ref_vertices"], f32)
    shared["verts"] = verts

    src = np.asarray(inputs["adj_src"]).astype(np.int64)
    dst = np.asarray(inputs["adj_dst"]).astype(np.int64)
    w = np.asarray(inputs["adj_w"], f32)
    at = np.zeros((NT * 128, NP), f32)
    np.add.at(at, (src, dst), w)
    shared["at"] = at.reshape(NT, 128, NP).astype(ml_dtypes.bfloat16)

    lin0_W = np.asarray(inputs["lin0_W"], f32)
    w0vt = np.zeros((4, 1024), f32)
    w0vt[0:3] = lin0_W[:, :3].T
    shared["w0vt"] = w0vt
    shared["w0imgt"] = np.ascontiguousarray(lin0_W[:, 3:].T).reshape(
        16, 128, 1024)

    ind = np.zeros((128, 16), f32)
    for c in range(128):
        ind[c, c // 8] = 1.0
    shared["g8"] = ind / 8.0
    shared["g8t"] = np.ascontiguousarray(ind.T)
    shared["ident"] = np.eye(128, dtype=f32)
    shared["identr"] = np.eye(128, dtype=f32)

    vals = {"lin0_b": inputs["lin0_b"],
            "b0_pre_g": inputs["b0_pre_g"], "b0_pre_b": inputs["b0_pre_b"],
            "b0_lin1_b": inputs["b0_lin1_b"],
            "b0_n1_g": inputs["b0_n1_g"], "b0_n1_b": inputs["b0_n1_b"],
            "b0_conv_b": inputs["b0_conv_b"],
            "b0_n2_g": inputs["b0_n2_g"], "b0_n2_b": inputs["b0_n2_b"],
            "b0_lin2_b": inputs["b0_lin2_b"], "b0_skip_b": inputs["b0_skip_b"],
            "h1_b": inputs["h1_b"], "h2_b": inputs["h2_b"],
            "hn_g": inputs["hn_g"], "hn_b": inputs["hn_b"],
            "h3_b": inputs["h3_b"]}
    for i in range(L):
        for nm, key in (("pre_g", "blk_pre_g"), ("pre_b", "blk_pre_b"),
                        ("lin1_b", "blk_lin1_b"), ("n1_g", "blk_n1_g"),
                        ("n1_b", "blk_n1_b"), ("conv_b", "blk_conv_b"),
                        ("n2_g", "blk_n2_g"), ("n2_b", "blk_n2_b"),
                        ("lin2_b", "blk_lin2_b")):
            vals[f"blk_{nm}{i}"] = np.asarray(inputs[key])[i]
    prm = np.zeros((128, NSLOT), f32)
    for (name, t), pos in PIDX.items():
        vec = np.asarray(vals[name], f32).ravel()
        seg = vec[t * 128:(t + 1) * 128]
        prm[0:len(seg), pos] = seg
    shared["prm"] = prm

    shared["b0l1t"] = np.ascontiguousarray(
        np.asarray(inputs["b0_lin1_W"], f32).T).reshape(8, 128, H)
    shared["b0cw"] = np.ascontiguousarray(
        np.asarray(inputs["b0_conv_W"], f32)).reshape(2, 128, H)
    shared["b0l2t"] = np.ascontiguousarray(
        np.asarray(inputs["b0_lin2_W"], f32).T).reshape(2, 128, C)
    shared["b0skt"] = np.ascontiguousarray(
        np.asarray(inputs["b0_skip_W"], f32).T).reshape(8, 128, C)
    shared["bl1t"] = np.ascontiguousarray(
        np.asarray(inputs["blk_lin1_W"], f32).transpose(0, 2, 1)).reshape(
            L, 4, 128, H)
    shared["bcw"] = np.ascontiguousarray(
        np.asarray(inputs["blk_conv_W"], f32)).reshape(L, 2, 128, H)
    shared["bl2t"] = np.ascontiguousarray(
        np.asarray(inputs["blk_lin2_W"], f32).transpose(0, 2, 1)).reshape(
            L, 2, 128, C)
    shared["h1t"] = np.ascontiguousarray(
        np.asarray(inputs["h1_W"], f32).T).reshape(4, 128, 64)
    shared["h2t"] = np.ascontiguousarray(np.asarray(inputs["h2_W"], f32).T)
    h3t = np.zeros((32, 4), f32)
    h3t[:, 0:3] = np.asarray(inputs["h3_W"], f32).T
    shared["h3t"] = h3t

    img = np.asarray(inputs["image_resnet"], f32)
    in_maps = []
    for c in range(NCORES):
        m = dict(shared)
        loc = img[c * BLOC:(c + 1) * BLOC].T
        m["imgt"] = np.ascontiguousarray(loc).reshape(16, 128, BLOC)
        in_maps.append(m)
    return in_maps


_NC_CACHE = {}


def _get_nc(nreps=1, **kw):
    key = (nreps, tuple(sorted(kw.items())))
    if key not in _NC_CACHE:
        _NC_CACHE[key] = build(nreps, **kw)
    return _NC_CACHE[key]


def run_on_hw(inputs, nreps=1, **kw):
    nc = _get_nc(nreps, **kw)
    in_maps = _host_prep(inputs)
    res = run_bass_kernel_spmd(nc, in_maps, core_ids=list(range(NCORES)),
                               trace=False)
    return np.concatenate([res.results[c]["out"] for c in range(NCORES)],
                          axis=0)


def kernel(**inputs) -> np.ndarray:
    return run_on_hw(inputs, nreps=1)



# revision 38
# speedup vs baseline: 91.8566x; 91.8566x over previous
"""Trainium2 Bass kernel for nn_ClothGraphConvNetwork_MLPDecoder.

8 NeuronCores, data-parallel over batch (2 batches/core), no collectives.

v2 design (on top of v1's separable-lin0 / dense-adjacency structure):
- Lockstep batch-pair interleave: the two per-core batches are emitted
  phase-alternately so one batch's matmuls hide the other's GroupNorm
  chain latency (v1 lost ~390us/rep to PE idle gaps).
- Adjacency matmul in fp8e4m3 with DoubleRow perf mode (2 k-tiles per
  instruction), kt-outer loop so stationary sup tiles load 4x less often.
  adj_w = 1/8 is exact in e4m3; messages average 8 ways so quantization
  noise ~1%.
- b0 skip path computed from host-precomputed skW@W0v and skW@W0img:
  SU = (skW@W0v)@verts once per rep, per-batch part collapses to a
  per-channel bias (svb2) folded into the b0 lin2 evacuation.
- bf16 activations (x, y1, y2, xr, su) and bf16 weights: same PE rate as
  f32r, half the SBUF/DMA, 2-4x DVE rate on applies.
- Shortened GN chains: PSUM-direct reads, scalar_tensor_tensor fusions,
  negated-mean trick (beta = (-m)*a + b in one op).
- Engine tables route applies across ACT/DVE/GPSIMD (Pool was 2.6% busy
  in v1).
"""

import contextlib

import numpy as np
import ml_dtypes

import concourse.bass as bass
import concourse.tile as tile
from concourse import bacc, mybir
from concourse.bass_utils import run_bass_kernel_spmd

F32R = mybir.dt.float32r
F32 = mybir.dt.float32
BF16 = mybir.dt.bfloat16
FP8 = mybir.dt.float8e4
AF = mybir.ActivationFunctionType
ALU = mybir.AluOpType
DR = mybir.MatmulPerfMode.DoubleRow

B, N, DEG = 16, 1723, 8
C, L, H = 512, 5, 256
NP = 1724              # padded vertex count
NCORES = 8
BLOC = B // NCORES     # batches per core
NT = 14                # vertex 128-tiles (last has 59 real rows)
NPAIR = 7              # DoubleRow k-tile pairs
FCH = [(0, 432), (432, 432), (864, 432), (1296, 428)]


def _param_layout():
    items = [("lin0_b", 1024),
             ("b0_pre_g", 1024), ("b0_pre_b", 1024),
             ("b0_lin1_b", 256), ("b0_n1_g", 256), ("b0_n1_b", 256),
             ("b0_conv_b", 256), ("b0_n2_g", 256), ("b0_n2_b", 256),
             ("b0_sklin2_b", 512)]
    for i in range(L):
        items += [(f"blk_pre_g{i}", 512), (f"blk_pre_b{i}", 512),
                  (f"blk_lin1_b{i}", 256), (f"blk_n1_g{i}", 256),
                  (f"blk_n1_b{i}", 256), (f"blk_conv_b{i}", 256),
                  (f"blk_n2_g{i}", 256), (f"blk_n2_b{i}", 256),
                  (f"blk_lin2_b{i}", 512)]
    items += [("h1_b", 64), ("h2_b", 32), ("hn_g", 32), ("hn_b", 32),
              ("h3_b", 3)]
    idx = {}
    pos = 0
    for name, ln in items:
        for t in range((ln + 127) // 128):
            idx[(name, t)] = pos
            pos += 1
    return items, idx, pos


PARAM_ITEMS, PIDX, NSLOT = _param_layout()
PHASES = []
FUSE_MT = False

# engine tables for relu-affine applies: "a"=ACT, "v"=DVE, "g"=GPSIMD
ENG_XR = {0: ("v", "a", "v", "a"), 1: ("a", "v", "a", "v")}   # [b][ct]
ENG_GN2 = {0: ("v", "a"), 1: ("a", "v")}                      # [ct][half]
ENG_GN3 = {0: ("v", "a"), 1: ("a", "v")}                      # [dt][half]
ENG_X0R = ("a", "v")                                          # [b]
HALVES = [(0, 864), (864, 860)]                               # FCH-aligned


def build(nreps=1, fp8agg=True, xf32=True, dump=0):
    nc = bacc.Bacc("TRN2", target_bir_lowering=False, debug=False)
    PHASES.clear()
    AGG_DT = FP8 if fp8agg else BF16
    X_DT = F32R if xf32 else BF16

    def _mark(label):
        PHASES.append((label, nc.next_id()))

    d = {}

    def din(name, shape, dt):
        d[name] = nc.dram_tensor(name, list(shape), dt, kind="ExternalInput")

    din("verts", (4, NP), F32R)
    din("at", (NT, 128, NP), AGG_DT)
    din("w0vt", (4, 1024), F32R)
    din("swt", (4, 512), F32R)
    din("vbh", (128, 8, BLOC), F32)
    din("svbh", (128, 4, BLOC), F32)
    din("g8", (128, 16), F32)     # indicator / 8  (group-mean reduce)
    din("g8t", (16, 128), F32)    # 0/1 indicator transpose (broadcast)
    din("identb", (128, 128), BF16)
    if xf32:
        din("identr", (128, 128), F32R)
    din("prm", (128, NSLOT), F32)
    din("b0l1t", (8, 128, H), BF16)
    din("b0cw", (2, 128, H), BF16)
    din("b0l2t", (2, 128, C), F32R)
    din("bl1t", (L, 4, 128, H), BF16)
    din("bcw", (L, 2, 128, H), BF16)
    din("bl2t", (L, 2, 128, C), F32R)
    din("h1t", (4, 128, 64), X_DT)
    din("h2t", (64, 32), F32R)
    din("h3t", (32, 4), F32R)
    out_d = nc.dram_tensor("out", [BLOC, 3, N], F32, kind="ExternalOutput")
    dbg_d = None
    if dump:
        dbg_d = nc.dram_tensor("dbg", [16, 128, NP], BF16,
                               kind="ExternalOutput")

    with tile.TileContext(nc) as tc, contextlib.ExitStack() as ctx:
        cons = ctx.enter_context(tc.tile_pool(name="cons", bufs=1))
        ps = ctx.enter_context(tc.tile_pool(name="ps", bufs=6, space="PSUM"))
        psc = ctx.enter_context(tc.tile_pool(name="psc", bufs=2, space="PSUM"))
        sm = ctx.enter_context(tc.tile_pool(name="sm", bufs=2))
        xp = ctx.enter_context(tc.tile_pool(name="xp", bufs=8))
        yp = ctx.enter_context(tc.tile_pool(name="yp", bufs=4))
        supp = ctx.enter_context(tc.tile_pool(name="supp", bufs=2))
        xrp = ctx.enter_context(tc.tile_pool(name="xrp", bufs=8))
        wp = ctx.enter_context(tc.tile_pool(name="wp", bufs=2))

        EV = {"v": nc.vector, "g": nc.gpsimd}

        def apply_ra(e, dst, src, a_ap, b_ap):
            """dst = relu(a*src + b); a/b per-partition (p,1) APs."""
            if e == "a":
                nc.scalar.activation(dst, src, AF.Relu, bias=b_ap, scale=a_ap)
            else:
                EV[e].tensor_scalar(dst, src, a_ap, b_ap,
                                    op0=ALU.mult, op1=ALU.add)
                EV[e].tensor_scalar_max(dst, dst, 0.0)

        # ---- constants ----
        g8 = cons.tile([128, 16], F32)
        nc.sync.dma_start(g8[:], d["g8"].ap())
        g8t = cons.tile([16, 128], F32)
        nc.sync.dma_start(g8t[:], d["g8t"].ap())
        identb = cons.tile([128, 128], BF16)
        nc.sync.dma_start(identb[:], d["identb"].ap())
        identx = identb
        if xf32:
            identx = cons.tile([128, 128], F32R)
            nc.sync.dma_start(identx[:], d["identr"].ap())
        prm = cons.tile([128, NSLOT], F32)
        nc.sync.dma_start(prm[:], d["prm"].ap())
        verts = cons.tile([4, NP], F32R)
        nc.sync.dma_start(verts[:], d["verts"].ap())
        w0vt = cons.tile([4, 1024], F32R)
        nc.sync.dma_start(w0vt[:], d["w0vt"].ap())
        swt = cons.tile([4, 512], F32R)
        nc.sync.dma_start(swt[:], d["swt"].ap())
        asb = cons.tile([128, NT, NP], AGG_DT)
        for kt in range(NT):
            nc.sync.dma_start(asb[:, kt, :], d["at"].ap()[kt])
        b0l1 = cons.tile([128, 8, H], BF16)
        for kt in range(8):
            nc.sync.dma_start(b0l1[:, kt, :], d["b0l1t"].ap()[kt])
        cw0 = cons.tile([128, 2, H], BF16)
        for ct in range(2):
            nc.sync.dma_start(cw0[:, ct, :], d["b0cw"].ap()[ct])
        l2t0 = cons.tile([128, 2, C], F32R)
        for ct in range(2):
            nc.sync.dma_start(l2t0[:, ct, :], d["b0l2t"].ap()[ct])
        h1w = cons.tile([128, 4, 64], X_DT)
        for kt in range(4):
            nc.sync.dma_start(h1w[:, kt, :], d["h1t"].ap()[kt])
        h2w = cons.tile([64, 32], F32R)
        nc.sync.dma_start(h2w[:], d["h2t"].ap())
        h3w = cons.tile([32, 4], F32R)
        nc.sync.dma_start(h3w[:], d["h3t"].ap())
        eps = cons.tile([128, 1], F32)
        nc.vector.memset(eps[:], 1e-5)
        vb = cons.tile([128, 8, BLOC], F32, name="vb")
        nc.sync.dma_start(vb[:], d["vbh"].ap())
        svb2 = cons.tile([128, 4, BLOC], F32, name="svb2")
        nc.sync.dma_start(svb2[:], d["svbh"].ap())

        def P(name, t=0, parts=128, width=1):
            i = PIDX[(name, t)]
            return prm[0:parts, i:i + width]

        # ---- GN chain helpers ----
        def gn_chain8(st3, gname, bname, abtag):
            """Batched T=8 chain for b0gn1: st3 (128,8,2) = [mean, E2]."""
            G, T = 16, 8
            psg = psc.tile([16, 8, 2], F32, tag="psc", name="psg8")
            nc.tensor.matmul(psg[0:G, 0:T, :], g8[:, 0:G],
                             st3[:, 0:T, :], start=True, stop=True)
            pg = sm.tile([16, 8, 2], F32, tag="pg8", bufs=2, name="pg8")
            nc.vector.tensor_copy(pg[0:G, 0:T, :], psg[0:G, 0:T, :])
            t2 = sm.tile([16, 8], F32, tag="t28", bufs=2, name="t28")
            nc.vector.tensor_tensor(t2[0:G, 0:T], pg[0:G, 0:T, 0],
                                    pg[0:G, 0:T, 0], op=ALU.mult)
            nc.vector.tensor_tensor(t2[0:G, 0:T], pg[0:G, 0:T, 1],
                                    t2[0:G, 0:T], op=ALU.subtract)
            nc.scalar.activation(t2[0:G, 0:T], t2[0:G, 0:T], AF.Sqrt,
                                 bias=eps[0:G, :])
            mr = sm.tile([16, 8, 2], F32, tag="mr8", bufs=2, name="mr8")
            nc.vector.tensor_copy(mr[0:G, 0:T, 0], pg[0:G, 0:T, 0])
            nc.vector.reciprocal(mr[0:G, 0:T, 1], t2[0:G, 0:T])
            psb = psc.tile([128, 8, 2], F32, tag="psc", name="psb8")
            nc.tensor.matmul(psb[:, 0:T, :], g8t[0:G, :],
                             mr[0:G, 0:T, :], start=True, stop=True)
            ab = sm.tile([128, 8, 2], F32, tag=abtag, bufs=2, name="ab8")
            nc.vector.tensor_tensor(ab[:, 0:T, 0], psb[:, 0:T, 1],
                                    P(gname, 0, 128, T), op=ALU.mult)
            t3 = sm.tile([128, 8], F32, tag="t38", bufs=2, name="t38")
            nc.vector.tensor_tensor(t3[:, 0:T], psb[:, 0:T, 0],
                                    ab[:, 0:T, 0], op=ALU.mult)
            nc.vector.tensor_tensor(ab[:, 0:T, 1], P(bname, 0, 128, T),
                                    t3[:, 0:T], op=ALU.subtract)
            return ab

        def stats_new(T):
            return [sm.tile([128, 4, 6], F32, tag="stats", bufs=14,
                            name="sts") for _ in range(T)]

        def note(stt, ci, x_ap, f0, fw, parts=128):
            rw = fw if f0 + fw <= N else (N - f0)
            nc.vector.bn_stats(stt[0:parts, ci, :],
                               x_ap[0:parts, f0:f0 + rw])

        def gn_f1a(stt, parts=128, G=16):
            """aggregate -> group mean/rstd; returns mr (G,2) = [-m_g, rs_g]."""
            st = sm.tile([128, 1, 2], F32, tag="st1", bufs=8, name="st1")
            nc.vector.bn_aggr(st[0:parts, 0, :], stt[0:parts, :, :])
            # E2 = m*m + v
            nc.vector.scalar_tensor_tensor(
                st[0:parts, 0, 1:2], st[0:parts, 0, 0:1],
                st[0:parts, 0, 0:1], st[0:parts, 0, 1:2],
                op0=ALU.mult, op1=ALU.add)
            psg = psc.tile([16, 2], F32, tag="psc", name="psg1")
            nc.tensor.matmul(psg[0:G, :], g8[0:parts, 0:G],
                             st[0:parts, 0, :], start=True, stop=True)
            t2 = sm.tile([16, 2], F32, tag="t21", bufs=8, name="t21")
            nc.vector.tensor_scalar(t2[0:G, 0:1], psg[0:G, 0:1],
                                    psg[0:G, 0:1], None, op0=ALU.mult)
            # (E2 + eps) - m^2
            nc.vector.scalar_tensor_tensor(
                t2[0:G, 1:2], psg[0:G, 1:2], 1e-5, t2[0:G, 0:1],
                op0=ALU.add, op1=ALU.subtract)
            nc.scalar.activation(t2[0:G, 1:2], t2[0:G, 1:2], AF.Sqrt)
            mr = sm.tile([16, 2], F32, tag="mr1", bufs=8, name="mr1")
            nc.vector.reciprocal(mr[0:G, 1:2], t2[0:G, 1:2])
            nc.vector.tensor_scalar(mr[0:G, 0:1], psg[0:G, 0:1], -1.0, None,
                                    op0=ALU.mult)
            return mr

        def gn_f1b(mr, gname, gt, bname, parts=128, G=16,
                   abtag="ab", abbufs=6):
            psb = psc.tile([128, 2], F32, tag="psc", name="psb1")
            nc.tensor.matmul(psb[0:parts, :], g8t[0:G, 0:parts], mr[0:G, :],
                             start=True, stop=True)
            ab = sm.tile([128, 2], F32, tag=abtag, bufs=abbufs, name="ab1")
            nc.vector.tensor_scalar(ab[0:parts, 0:1], psb[0:parts, 1:2],
                                    P(gname, gt, parts), None, op0=ALU.mult)
            # beta = (-m)*a + b
            nc.vector.scalar_tensor_tensor(
                ab[0:parts, 1:2], psb[0:parts, 0:1], ab[0:parts, 0:1],
                P(bname, gt, parts), op0=ALU.mult, op1=ALU.add)
            return ab

        # ================= phase bodies =================

        def lin1_pair(pn, x, xab, klist):
            """y1[b] = lin1(relu-affine(x[b])) for both batches, lockstep.
            Per ci, batch A's evac+stats are emitted before batch B's
            matmuls so A's gn2 chain deps complete while B runs on PE."""
            y1 = {b: [yp.tile([128, NP], BF16, tag="y1", name="y1")
                      for _ in range(2)] for b in (0, 1)}
            ystats = {b: stats_new(2) for b in (0, 1)}
            for ci, (f0, fw) in enumerate(FCH):
                y1ps = {b: [ps.tile([128, 512], F32, tag="ps", name="y1ps")
                            for _ in range(2)] for b in (0, 1)}
                xr = {}
                for b in (0, 1):
                    for ct in range(4):
                        xr[(b, ct)] = xrp.tile([128, 432], BF16, tag="xr",
                                               name="xr")
                        apply_ra(ENG_XR[b][ct], xr[(b, ct)][:, :fw],
                                 x[b][ct][:, f0:f0 + fw],
                                 xab[b][ct][:, 0:1], xab[b][ct][:, 1:2])
                for b in (0, 1):
                    for ct in range(4):
                        for mt in range(2):
                            nc.tensor.matmul(
                                y1ps[b][mt][:, :fw],
                                klist(ct)[:, mt * 128:(mt + 1) * 128],
                                xr[(b, ct)][:, :fw],
                                start=(ct == 0), stop=(ct == 3))
                    for mt in range(2):
                        nc.scalar.activation(y1[b][mt][:, f0:f0 + fw],
                                             y1ps[b][mt][:, :fw], AF.Identity,
                                             bias=P(pn("lin1_b"), mt))
                        note(ystats[b][mt], ci, y1[b][mt], f0, fw)
            return y1, ystats

        def tail_pair(pn, cw, l2w, xsrc, xdst, l2bias, y1, ystats,
                      pn_next, idm=None, post=None):
            """gn2 -> sup -> agg -> gn3 -> lin2 (+residual) for both batches.

            xsrc(b, mt) -> (128, NP) AP read for residual; xdst[b][mt] tiles
            written by lin2 evac with bias l2bias(b, mt)."""
            # --- gn2 + sup ---
            _mark("gn2")
            # all four chains first: their tiny PE matmuls sit right after
            # both batches' lin1 waves, deps long since complete
            ab2 = {}
            for b in (0, 1):
                ab2[b] = [gn_f1b(gn_f1a(ystats[b][ct]),
                                 pn("n1_g"), ct, pn("n1_b"))
                          for ct in range(2)]
            _mark("sup")
            sup = {}

            def sup_pairs(b, plist):
                for np_ in plist:
                    sps = ps.tile([128, 512], F32, tag="ps", name="sps")
                    for half in range(2):
                        nt = np_ * 2 + half
                        ms = nt * 128
                        mw = min(ms + 128, NP) - ms
                        for ct in range(2):
                            nc.tensor.matmul(
                                sps[0:mw, half * H:half * H + H],
                                y1[b][ct][:, ms:ms + mw], cw(ct),
                                start=(ct == 0), stop=(ct == 1))
                    for half in range(2):
                        nt = np_ * 2 + half
                        ms = nt * 128
                        mw = min(ms + 128, NP) - ms
                        if half == 0:
                            nc.scalar.copy(sup[b][0:mw, nt, :],
                                           sps[0:mw, 0:H])
                        else:
                            nc.vector.tensor_copy(sup[b][0:mw, nt, :],
                                                  sps[0:mw, H:2 * H])

            for b in (0, 1):
                sup[b] = supp.tile([128, NT, H], BF16, tag="sup",
                                   name="sup")
                # apply gn2 half 0 -> sup pairs 0-2, half 1 -> pairs 3-6
                for hi, (h0, hw) in enumerate(HALVES):
                    for ct in range(2):
                        apply_ra(ENG_GN2[ct][hi],
                                 y1[b][ct][:, h0:h0 + hw],
                                 y1[b][ct][:, h0:h0 + hw],
                                 ab2[b][ct][:, 0:1], ab2[b][ct][:, 1:2])
                    sup_pairs(b, range(0, 3) if hi == 0 else range(3, NPAIR))

            # --- agg (adjacency matmul) ---
            _mark("agg")
            y2 = {}
            y2stats = {}
            for b in (0, 1):
                y2[b] = [yp.tile([128, NP], F32R, tag="y1", name="y2")
                         for _ in range(2)]
                y2stats[b] = stats_new(2)
                for dt in range(2):
                    aps = [ps.tile([128, 512], F32, tag="ps", name="aps")
                           for _ in range(4)]
                    for kt in range(NT):
                        kn = min(128, N - kt * 128)
                        for ci, (f0, fw) in enumerate(FCH):
                            nc.tensor.matmul(
                                aps[ci][:, :fw],
                                sup[b][0:kn, kt,
                                       dt * 128:(dt + 1) * 128],
                                asb[0:kn, kt, f0:f0 + fw],
                                start=(kt == 0), stop=(kt == NT - 1))
                    for ci, (f0, fw) in enumerate(FCH):
                        nc.scalar.activation(y2[b][dt][:, f0:f0 + fw],
                                             aps[ci][:, :fw], AF.Identity,
                                             bias=P(pn("conv_b"), dt))
                        note(y2stats[b][dt], ci, y2[b][dt], f0, fw)
            _mark("gn3")
            mr3 = {b: [gn_f1a(y2stats[b][dt]) for dt in range(2)]
                   for b in (0, 1)}

            # --- gn3 apply + lin2 + residual ---
            _mark("lin2")
            collect = pn_next is not None
            xstats = {}
            mrx = {0: {}, 1: {}}
            for b in (0, 1):
                for dt in range(2):
                    ab = gn_f1b(mr3[b][dt], pn("n2_g"), dt, pn("n2_b"))
                    for hi, (h0, hw) in enumerate(HALVES):
                        apply_ra(ENG_GN3[dt][hi], y2[b][dt][:, h0:h0 + hw],
                                 y2[b][dt][:, h0:h0 + hw],
                                 ab[:, 0:1], ab[:, 1:2])
                xstats[b] = stats_new(4) if collect else None
                lps = {}

                def ident_mm(mt, ci):
                    if FUSE_MT and mt % 2 == 1:
                        return   # fused residual: no identity matmul
                    f0, fw = FCH[ci]
                    lps[(mt, ci)] = ps.tile([128, 512], F32, tag="ps",
                                            name="lps")
                    nc.tensor.matmul(lps[(mt, ci)][:, :fw], idm[:],
                                     xsrc(b, mt)[:, f0:f0 + fw],
                                     start=True, stop=False)

                ident_mm(0, 0)
                ident_mm(0, 1)
                for mt in range(4):
                    fuse = FUSE_MT and mt % 2 == 1
                    for ci, (f0, fw) in enumerate(FCH):
                        # keep the ident pipeline two chunks ahead
                        if ci < 2:
                            ident_mm(mt, ci + 2)
                        elif mt < 3:
                            ident_mm(mt + 1, ci - 2)
                        if fuse:
                            lps[(mt, ci)] = ps.tile([128, 512], F32,
                                                    tag="ps", name="lps")
                        for ct in range(2):
                            nc.tensor.matmul(lps[(mt, ci)][:, :fw],
                                             l2w(ct, mt),
                                             y2[b][ct][:, f0:f0 + fw],
                                             start=(fuse and ct == 0),
                                             stop=(ct == 1))
                        if fuse:
                            nc.vector.scalar_tensor_tensor(
                                xdst[b][mt][:, f0:f0 + fw],
                                lps.pop((mt, ci))[:, :fw], l2bias(b, mt),
                                xsrc(b, mt)[:, f0:f0 + fw],
                                op0=ALU.add, op1=ALU.add)
                        else:
                            nc.scalar.activation(xdst[b][mt][:, f0:f0 + fw],
                                                 lps.pop((mt, ci))[:, :fw],
                                                 AF.Identity,
                                                 bias=l2bias(b, mt))
                        if collect:
                            note(xstats[b][mt], ci, xdst[b][mt], f0, fw)
                    if collect and mt >= 1:
                        mrx[b][mt - 1] = gn_f1a(xstats[b][mt - 1])
                if post is not None:
                    post(b)
            if not collect:
                return None
            mrx[0][3] = gn_f1a(xstats[0][3])
            mrx[1][3] = gn_f1a(xstats[1][3])
            xab = {}
            for b in (0, 1):
                xab[b] = [gn_f1b(mrx[b][mt], pn_next("pre_g"), mt,
                                 pn_next("pre_b"), abtag="abx", abbufs=8)
                          for mt in range(4)]
            return xab

        # ================= program =================
        rep = tc.For_i(0, nreps, 1) if nreps > 1 else contextlib.nullcontext()
        with rep:
            _mark("setup")
            # U per-channel stats (batch-independent)
            uch = cons.tile([128, 8, 2], F32, name="uch")
            for kt in range(8):
                ust = sm.tile([128, 4, 6], F32, tag="stats", bufs=14,
                              name="ust")
                for ci, (f0, fw) in enumerate(FCH):
                    ups = ps.tile([128, 512], F32, tag="ps", name="ups")
                    nc.tensor.matmul(ups[:, :fw],
                                     w0vt[:, kt * 128:(kt + 1) * 128],
                                     verts[:, f0:f0 + fw], start=True,
                                     stop=True)
                    rw = fw if f0 + fw <= N else (N - f0)
                    nc.vector.bn_stats(ust[:, ci, :], ups[:, 0:rw])
                nc.vector.bn_aggr(uch[:, kt, :], ust[:, :, :])
            # ---- b0 GN1: analytic coefficients per batch ----
            _mark("b0gn1")
            ab0 = {}
            for b in (0, 1):
                st3 = sm.tile([128, 8, 2], F32, tag="st3b", bufs=2,
                              name="st3b")
                sq = sm.tile([128, 8], F32, tag="sqb", bufs=2, name="sqb")
                nc.vector.tensor_tensor(st3[:, :, 0], uch[:, :, 0],
                                        vb[:, :, b], op=ALU.add)
                nc.vector.tensor_tensor(sq[:, :], st3[:, :, 0], st3[:, :, 0],
                                        op=ALU.mult)
                nc.vector.tensor_tensor(st3[:, :, 1], uch[:, :, 1], sq[:, :],
                                        op=ALU.add)
                ab = gn_chain8(st3, "b0_pre_g", "b0_pre_b",
                               abtag=f"ab0_{b}")
                t5 = sm.tile([128, 8], F32, tag="t5", bufs=2, name="t5")
                nc.vector.tensor_tensor(t5[:, :], ab[:, :, 0],
                                        vb[:, :, b], op=ALU.mult)
                nc.vector.tensor_tensor(ab[:, :, 1], ab[:, :, 1],
                                        t5[:, :], op=ALU.add)
                ab0[b] = ab

            # SU = (skW @ W0v) @ verts — emitted after the b0gn1 chains:
            # its PE matmuls cover the chains' DVE latency
            su = cons.tile([128, 4, NP], BF16, name="su")
            for ci, (f0, fw) in enumerate(FCH):
                for mt in range(4):
                    sps = ps.tile([128, 512], F32, tag="ps", name="spsu")
                    nc.tensor.matmul(sps[:, :fw],
                                     swt[:, mt * 128:(mt + 1) * 128],
                                     verts[:, f0:f0 + fw], start=True,
                                     stop=True)
                    nc.scalar.copy(su[:, mt, f0:f0 + fw], sps[:, :fw])

            # ---- b0 front: joint over batches (shared U chunks) ----
            _mark("b0front")
            y1 = {b: [yp.tile([128, NP], BF16, tag="y1", name="y1f")
                      for _ in range(2)] for b in (0, 1)}
            ystats = {b: stats_new(2) for b in (0, 1)}
            for ci, (f0, fw) in enumerate(FCH):
                y1ps = {b: [ps.tile([128, 512], F32, tag="ps", name="y1psf")
                            for _ in range(2)] for b in (0, 1)}

                def u_mm(kt):
                    ups = ps.tile([128, 512], F32, tag="ps", name="ups2")
                    nc.tensor.matmul(ups[:, :fw],
                                     w0vt[:, kt * 128:(kt + 1) * 128],
                                     verts[:, f0:f0 + fw],
                                     start=True, stop=True)
                    return ups

                ups = u_mm(0)   # one kt ahead so applies hide under PE work
                for kt in range(8):
                    x0r = {}
                    for b in (0, 1):
                        x0r[b] = xrp.tile([128, 432], BF16, tag="xr",
                                          name="x0r")
                        apply_ra(ENG_X0R[b], x0r[b][:, :fw], ups[:, :fw],
                                 ab0[b][:, kt, 0:1], ab0[b][:, kt, 1:2])
                    if kt < 7:
                        ups = u_mm(kt + 1)
                    for b in (0, 1):
                        for mt in range(2):
                            nc.tensor.matmul(
                                y1ps[b][mt][:, :fw],
                                b0l1[:, kt, mt * 128:(mt + 1) * 128],
                                x0r[b][:, :fw],
                                start=(kt == 0), stop=(kt == 7))
                for b in (0, 1):
                    for mt in range(2):
                        nc.scalar.activation(y1[b][mt][:, f0:f0 + fw],
                                             y1ps[b][mt][:, :fw], AF.Identity,
                                             bias=P("b0_lin1_b", mt))
                        note(ystats[b][mt], ci, y1[b][mt], f0, fw)

            if dump == 2:
                for b in (0, 1):
                    for mt in range(2):
                        nc.sync.dma_start(dbg_d.ap()[b * 4 + mt],
                                          y1[b][mt][:, :])

            # ---- b0 tail ----
            x = {b: [xp.tile([128, NP], X_DT, tag="x", name="x")
                     for _ in range(4)] for b in (0, 1)}
            xab = tail_pair(
                lambda s: "b0_" + s,
                lambda ct: cw0[:, ct, :],
                lambda ct, mt: l2t0[:, ct, mt * 128:(mt + 1) * 128],
                lambda b, mt: su[:, mt, :],
                x,
                lambda b, mt: svb2[:, mt, b:b + 1],
                y1, ystats, pn_next=lambda s: f"blk_{s}0", idm=identb)

            if dump == 1:
                for b in (0, 1):
                    for mt in range(4):
                        nc.sync.dma_start(dbg_d.ap()[b * 4 + mt],
                                          x[b][mt][:, :])

            # ---- head: stage 1 (h1/h2+stats) rides each batch's lin2;
            # stage 2 (chain+apply+h3+out) for both batches at the end ----
            hd = {}

            def emit_head(b):
                _mark("head")
                yh1 = yp.tile([64, NP], F32R, tag="y1", name="yh1")
                for (f0, fw) in FCH:
                    hps = ps.tile([64, 512], F32, tag="ps", name="hps")
                    for kt in range(4):
                        nc.tensor.matmul(hps[:, :fw], h1w[:, kt, :],
                                         x[b][kt][:, f0:f0 + fw],
                                         start=(kt == 0), stop=(kt == 3))
                    nc.scalar.activation(yh1[:, f0:f0 + fw], hps[:, :fw],
                                         AF.Relu, bias=P("h1_b", 0, 64))
                yh2 = yp.tile([32, NP], F32R, tag="y1", name="yh2")
                hstats = stats_new(1)
                for ci, (f0, fw) in enumerate(FCH):
                    hps2 = ps.tile([32, 512], F32, tag="ps", name="hps2")
                    nc.tensor.matmul(hps2[:, :fw], h2w[:], yh1[:, f0:f0 + fw],
                                     start=True, stop=True)
                    nc.scalar.activation(yh2[:, f0:f0 + fw], hps2[:, :fw],
                                         AF.Identity, bias=P("h2_b", 0, 32))
                    note(hstats[0], ci, yh2, f0, fw, parts=32)
                hd[b] = (yh2, hstats)

            def emit_head2(b):
                _mark("head")
                yh2, hstats = hd[b]
                mr = gn_f1a(hstats[0], parts=32, G=4)
                abh = gn_f1b(mr, "hn_g", 0, "hn_b", parts=32, G=4)
                for ci, (f0, fw) in enumerate(FCH):
                    apply_ra("v" if ci % 2 else "a", yh2[:, f0:f0 + fw],
                             yh2[:, f0:f0 + fw],
                             abh[0:32, 0:1], abh[0:32, 1:2])
                osb = wp.tile([4, NP], F32, tag="osb", bufs=2, name="osb")
                for (f0, fw) in FCH:
                    hps3 = ps.tile([4, 512], F32, tag="ps", name="hps3")
                    nc.tensor.matmul(hps3[:, :fw], h3w[:],
                                     yh2[:, f0:f0 + fw],
                                     start=True, stop=True)
                    nc.scalar.activation(osb[0:3, f0:f0 + fw],
                                         hps3[0:3, :fw],
                                         AF.Identity, bias=P("h3_b", 0, 3))
                nc.sync.dma_start(out_d.ap()[b], osb[0:3, 0:N])

            # ---- 5 residual blocks ----
            bw = {}

            def load_blk(i):
                bl1 = wp.tile([128, 4, H], BF16, tag="bl1", name="bl1")
                for ct in range(4):
                    nc.sync.dma_start(bl1[:, ct, :], d["bl1t"].ap()[i, ct])
                bcw = wp.tile([128, 2, H], BF16, tag="bcw", name="bcw")
                for ct in range(2):
                    nc.sync.dma_start(bcw[:, ct, :], d["bcw"].ap()[i, ct])
                bl2 = wp.tile([128, 2, C], F32R, tag="bl2", name="bl2")
                for ct in range(2):
                    nc.sync.dma_start(bl2[:, ct, :], d["bl2t"].ap()[i, ct])
                bw[i] = (bl1, bcw, bl2)

            load_blk(0)
            for i in range(L):
                bl1, bcw, bl2 = bw.pop(i)
                _mark("lin1")
                y1, ystats = lin1_pair(
                    lambda s, i=i: f"blk_{s}{i}", x, xab,
                    lambda ct, _w=bl1: _w[:, ct, :])
                if dump == 3 and i == 0:
                    for b in (0, 1):
                        for mt in range(2):
                            nc.sync.dma_start(dbg_d.ap()[b * 4 + mt],
                                              y1[b][mt][:, :])
                if i + 1 < L:
                    load_blk(i + 1)
                pn_next = (lambda s, j=i + 1: f"blk_{s}{j}") \
                    if i < L - 1 else None
                xab = tail_pair(
                    lambda s, i=i: f"blk_{s}{i}",
                    lambda ct, _w=bcw: _w[:, ct, :],
                    lambda ct, mt, _w=bl2: _w[:, ct, mt * 128:(mt + 1) * 128],
                    lambda b, mt: x[b][mt][:, :],
                    x,
                    lambda b, mt, i=i: P(f"blk_lin2_b{i}", mt),
                    y1, ystats, pn_next=pn_next, idm=identx,
                    post=emit_head if i == L - 1 else None)

            emit_head2(0)
            emit_head2(1)


    nc.compile()
    return nc


def _host_prep(inputs, fp8agg=True, xf32=True):
    f32 = np.float32
    bf = ml_dtypes.bfloat16
    shared = {}

    verts = np.zeros((4, NP), f32)
    verts[0:3, 0:N] = np.asarray(inputs["ref_vertices"], f32)
    shared["verts"] = verts

    src = np.asarray(inputs["adj_src"]).astype(np.int64)
    dst = np.asarray(inputs["adj_dst"]).astype(np.int64)
    w = np.asarray(inputs["adj_w"], f32)
    at = np.zeros((NT * 128, NP), f32)
    np.add.at(at, (src, dst), w)
    adt = ml_dtypes.float8_e4m3fn if fp8agg else bf
    shared["at"] = at.reshape(NT, 128, NP).astype(adt)

    lin0_W = np.asarray(inputs["lin0_W"], f32)
    skW = np.asarray(inputs["b0_skip_W"], f32)
    w0vt = np.zeros((4, 1024), f32)
    w0vt[0:3] = lin0_W[:, :3].T
    shared["w0vt"] = w0vt
    swt = np.zeros((4, 512), f32)
    swt[0:3] = (skW @ lin0_W[:, :3]).T
    shared["swt"] = swt

    ind = np.zeros((128, 16), f32)
    for c in range(128):
        ind[c, c // 8] = 1.0
    shared["g8"] = ind / 8.0
    shared["g8t"] = np.ascontiguousarray(ind.T)
    xdt = f32 if xf32 else bf
    shared["identb"] = np.eye(128).astype(bf)
    if xf32:
        shared["identr"] = np.eye(128, dtype=f32)

    sklin2_b = (skW @ np.asarray(inputs["lin0_b"], f32)
                + np.asarray(inputs["b0_skip_b"], f32)
                + np.asarray(inputs["b0_lin2_b"], f32))
    vals = {"lin0_b": inputs["lin0_b"],
            "b0_pre_g": inputs["b0_pre_g"], "b0_pre_b": inputs["b0_pre_b"],
            "b0_lin1_b": inputs["b0_lin1_b"],
            "b0_n1_g": inputs["b0_n1_g"], "b0_n1_b": inputs["b0_n1_b"],
            "b0_conv_b": inputs["b0_conv_b"],
            "b0_n2_g": inputs["b0_n2_g"], "b0_n2_b": inputs["b0_n2_b"],
            "b0_sklin2_b": sklin2_b,
            "h1_b": inputs["h1_b"], "h2_b": inputs["h2_b"],
            "hn_g": inputs["hn_g"], "hn_b": inputs["hn_b"],
            "h3_b": inputs["h3_b"]}
    for i in range(L):
        for nm, key in (("pre_g", "blk_pre_g"), ("pre_b", "blk_pre_b"),
                        ("lin1_b", "blk_lin1_b"), ("n1_g", "blk_n1_g"),
                        ("n1_b", "blk_n1_b"), ("conv_b", "blk_conv_b"),
                        ("n2_g", "blk_n2_g"), ("n2_b", "blk_n2_b"),
                        ("lin2_b", "blk_lin2_b")):
            vals[f"blk_{nm}{i}"] = np.asarray(inputs[key])[i]
    prm = np.zeros((128, NSLOT), f32)
    for (name, t), pos in PIDX.items():
        vec = np.asarray(vals[name], f32).ravel()
        seg = vec[t * 128:(t + 1) * 128]
        prm[0:len(seg), pos] = seg
    shared["prm"] = prm

    shared["b0l1t"] = np.ascontiguousarray(
        np.asarray(inputs["b0_lin1_W"], f32).T).reshape(8, 128, H).astype(bf)
    shared["b0cw"] = np.ascontiguousarray(
        np.asarray(inputs["b0_conv_W"], f32)).reshape(2, 128, H).astype(bf)
    shared["b0l2t"] = np.ascontiguousarray(
        np.asarray(inputs["b0_lin2_W"], f32).T).reshape(2, 128, C)
    shared["bl1t"] = np.ascontiguousarray(
        np.asarray(inputs["blk_lin1_W"], f32).transpose(0, 2, 1)).reshape(
            L, 4, 128, H).astype(bf)
    shared["bcw"] = np.ascontiguousarray(
        np.asarray(inputs["blk_conv_W"], f32)).reshape(L, 2, 128, H).astype(bf)
    shared["bl2t"] = np.ascontiguousarray(
        np.asarray(inputs["blk_lin2_W"], f32).transpose(0, 2, 1)).reshape(
            L, 2, 128, C)
    shared["h1t"] = np.ascontiguousarray(
        np.asarray(inputs["h1_W"], f32).T).reshape(4, 128, 64).astype(xdt)
    shared["h2t"] = np.ascontiguousarray(np.asarray(inputs["h2_W"], f32).T)
    h3t = np.zeros((32, 4), f32)
    h3t[:, 0:3] = np.asarray(inputs["h3_W"], f32).T
    shared["h3t"] = h3t

    img = np.asarray(inputs["image_resnet"], f32)
    lin0_b = np.asarray(inputs["lin0_b"], f32)
    vb_all = lin0_W[:, 3:] @ img.T + lin0_b[:, None]       # (1024, B)
    svb_all = skW @ (lin0_W[:, 3:] @ img.T) + sklin2_b[:, None]  # (512, B)
    in_maps = []
    for c in range(NCORES):
        m = dict(shared)
        vb_c = vb_all[:, c * BLOC:(c + 1) * BLOC]
        m["vbh"] = np.ascontiguousarray(
            vb_c.reshape(8, 128, BLOC).transpose(1, 0, 2))
        svb_c = svb_all[:, c * BLOC:(c + 1) * BLOC]
        m["svbh"] = np.ascontiguousarray(
            svb_c.reshape(4, 128, BLOC).transpose(1, 0, 2))
        in_maps.append(m)
    return in_maps


_NC_CACHE = {}


def _get_nc(nreps=1, **kw):
    key = (nreps, tuple(sorted(kw.items())))
    if key not in _NC_CACHE:
        _NC_CACHE[key] = build(nreps, **kw)
    return _NC_CACHE[key]


def run_on_hw(inputs, nreps=1, **kw):
    nc = _get_nc(nreps, **kw)
    in_maps = _host_prep(inputs, fp8agg=kw.get("fp8agg", True),
                         xf32=kw.get("xf32", True))
    res = run_bass_kernel_spmd(nc, in_maps, core_ids=list(range(NCORES)),
                               trace=False)
    return np.concatenate([res.results[c]["out"] for c in range(NCORES)],
                          axis=0)


def run_dbg(inputs, dump, **kw):
    nc = _get_nc(1, dump=dump, **kw)
    in_maps = _host_prep(inputs, fp8agg=kw.get("fp8agg", True),
                         xf32=kw.get("xf32", True))
    res = run_bass_kernel_spmd(nc, in_maps, core_ids=list(range(NCORES)),
                               trace=False)
    return res.results[0]["dbg"]


def kernel(**inputs) -> np.ndarray:
    return run_on_hw(inputs, nreps=1)
